# revision 1
# baseline (speedup 1.0000x reference)
"""Trainium2 Bass kernel for nn_CrossAttnHead (cross-attention head + FFN).

Math (reference):
  Q = concat(A bcast over t, phi_tar bcast over (b,h)) @ Wq^T + bq
  K,V = H_emb_obs @ {Wk,Wv}^T + b
  scores = (Qh . Kh)/sqrt(dh) ; attn = softmax(scores, axis=o)
  ctx = attn @ Vh ; y = Linear2(relu(LN(Linear1(ctx))))

Key structure exploited on device:
  Q[b,h,t] = QA[b,h] + Qphi[t]  (concat-linear splits into two small matmuls)
  => scores[b,h,n,t,o] = SA[b,h,n,o] + Sphi[b,n,t,o]
  => exp(scores/s) = w[b,h,n,o] * U[b,n,t,o],  w = exp(SA/s), U = exp(Sphi/s)
  so softmax numerator/denominator come from one matmul of U against
  w-scaled V (with a ones-column appended for the denominator).

Sharding: data-parallel over B: 16 batches -> 8 cores x 2 batches.
"""

import os
import numpy as np

import concourse.bass as bass
import concourse.mybir as mybir
import concourse.tile as tile
from concourse.bass_utils import run_bass_kernel_spmd
from concourse.masks import make_identity
from concourse.vector_clock import ScopedClock
import bass_rust

# ---- problem constants (hardcoded per contract) ----
B, H, NTAR, NOBS = 16, 12, 256, 512
DSITE = 128          # d (site embedding)
DTOT = 128           # d_tot == h_temporal
M_BASIS = 128        # phi basis dim
NH, DH = 4, 32       # heads
NCORES = 8
BPC = B // NCORES    # batches per core = 2
SCALE = 1.0 / np.sqrt(DH)
LN_EPS = 1e-5

f32 = mybir.dt.float32
AF = mybir.ActivationFunctionType
ALU = mybir.AluOpType

OC = 4               # NOBS / 128 chunks
TC = 2               # NTAR / 128 chunks


# ------------------------------------------------------------------
# walrus in this container rejects >1 sem wait per instruction (CTRL Drain,
# matmul LDWEIGHTS struct, ...). Two patches:
#  1) _add_instruction: hoist all-but-one waits of any instruction onto
#     same-engine NoOps inserted just before it (sequencer semantics are
#     identical: all waits must pass before the inst dispatches).
#  2) _drain_and_barrier: the tail drain gets its waits added after
#     insertion, so split it into one drain per wait.
def _install_drain_patch():
    _orig_add = tile.TileContext._add_instruction

    def _add_split(self, inst):
        si = getattr(inst, "sync_info", None)
        if si is not None and si.on_wait and len(si.on_wait) > 1:
            waits = list(si.on_wait)
            si.on_wait = waits[-1:]
            for w in waits[:-1]:
                nop = mybir.InstNoOp(
                    name=self.nc.get_next_instruction_name(),
                    sync_info=mybir.SyncInfo(on_wait=[w], on_update=[]),
                    bass_nofuse=True,
                    engine=inst.engine,
                )
                _orig_add(self, nop)
        _orig_add(self, inst)

    tile.TileContext._add_instruction = _add_split

    def _patched(self, tick_clock, wait_clock):
        d0 = self.nc.sync.drain()
        wait_clock.add_sem_waits(
            d0.ins, ScopedClock({None: tick_clock.global_clock})
        )
        si = d0.ins.sync_info
        if si is not None and si.on_wait and len(si.on_wait) > 1:
            waits = list(si.on_wait)
            si.on_wait = waits[:1]
            for w in waits[1:]:
                d = self.nc.sync.drain()
                dsi = d.ins.sync_info
                if dsi is None:
                    d.ins.sync_info = bass_rust.SyncInfo(
                        on_wait=[w], on_update=[]
                    )
                else:
                    dsi.on_wait = [w]
        self.nc.all_engine_barrier()
        popped = self.nc._tile_sem_poison_stack.pop()
        assert popped is self._sem_poison
        # chunk the sem range-clears: wide EVENT_SEMAPHORE_RANGE_CLEAR
        # trips "ISA wrong length" in this walrus build
        sems = list(self.sems.allocated().values())
        for i in range(0, len(sems), 4):
            self.nc.clear_and_free_semaphores(sems[i : i + 4])
        self.nc.all_engine_barrier()

    tile.TileContext._drain_and_barrier = _patched


_install_drain_patch()


def _bc(ap, ap_dims):
    """AP with the same tensor/offset but explicit [step, count] dims."""
    return bass.AP(tensor=ap.tensor, offset=ap.offset, ap=ap_dims)


def build_nc(ln_general: bool):
    nc = bass.Bass()
    dt = f32

    # ---- DRAM I/O ----
    AT = nc.dram_tensor("AT", [128, BPC * H], dt, kind="ExternalInput")
    Hemb = nc.dram_tensor("Hemb", [BPC, NOBS, DSITE], dt, kind="ExternalInput")
    phiT = nc.dram_tensor("phiT", [128, NTAR], dt, kind="ExternalInput")
    WqAT = nc.dram_tensor("WqAT", [128, 128], dt, kind="ExternalInput")
    WqPT = nc.dram_tensor("WqPT", [128, 128], dt, kind="ExternalInput")
    WkT = nc.dram_tensor("WkT", [128, 128], dt, kind="ExternalInput")
    WvT = nc.dram_tensor("WvT", [128, 128], dt, kind="ExternalInput")
    W1T = nc.dram_tensor("W1T", [128, 128], dt, kind="ExternalInput")
    bq_c = nc.dram_tensor("bq_c", [128, 1], dt, kind="ExternalInput")
    bk_c = nc.dram_tensor("bk_c", [128, 1], dt, kind="ExternalInput")
    bv_rep = nc.dram_tensor("bv_rep", [128, 128], dt, kind="ExternalInput")
    b1_rep = nc.dram_tensor("b1_rep", [128, 128], dt, kind="ExternalInput")
    W2_rep = nc.dram_tensor("W2_rep", [128, 128], dt, kind="ExternalInput")
    b2_t = nc.dram_tensor("b2_t", [1, 1], dt, kind="ExternalInput")
    if ln_general:
        g_rep_t = nc.dram_tensor("g_rep", [128, 128], dt, kind="ExternalInput")
        lb_rep_t = nc.dram_tensor("lb_rep", [128, 128], dt, kind="ExternalInput")
    Y = nc.dram_tensor("Y", [BPC * H * TC, 128], dt, kind="ExternalOutput")

    with tile.TileContext(nc) as tc:
        _emit(nc, tc, locals(), ln_general)
    return nc


def _emit(nc, tc, T, ln_general):
    dt = f32
    from contextlib import ExitStack

    ctxmgr = ExitStack()
    with ctxmgr:
        singles = ctxmgr.enter_context(tc.tile_pool(name="singles", bufs=1))
        sb_ht = ctxmgr.enter_context(tc.tile_pool(name="ht", bufs=2))
        sb_kt = ctxmgr.enter_context(tc.tile_pool(name="kt", bufs=2))
        sb_v1a = ctxmgr.enter_context(tc.tile_pool(name="v1a", bufs=3))
        sb_w = ctxmgr.enter_context(tc.tile_pool(name="wp", bufs=3))
        sb_vp = ctxmgr.enter_context(tc.tile_pool(name="vp", bufs=5))
        sb_ut = ctxmgr.enter_context(tc.tile_pool(name="ut", bufs=5))
        sb_x1 = ctxmgr.enter_context(tc.tile_pool(name="x1", bufs=3))
        sb_x1t = ctxmgr.enter_context(tc.tile_pool(name="x1t", bufs=3))
        sb_ffn = ctxmgr.enter_context(tc.tile_pool(name="ffn", bufs=3))
        # all 12 h-tiles of a (b,tc) group stay live until the batched LN
        # stats complete -> needs >=12 slots plus pipelining slack
        sb_x2s = ctxmgr.enter_context(tc.tile_pool(name="x2s", bufs=14))
        sb_stats = ctxmgr.enter_context(tc.tile_pool(name="stats", bufs=3))
        ps_u = ctxmgr.enter_context(tc.tile_pool(name="psu", bufs=2, space="PSUM"))
        ps_b = ctxmgr.enter_context(tc.tile_pool(name="psb", bufs=4, space="PSUM"))

        # ---- load constants ----
        def load(name, shape):
            t = singles.tile(shape, dt, tag=name)
            nc.sync.dma_start(out=t[:], in_=T[name][:])
            return t

        at_s = load("AT", [128, BPC * H])
        phiT_s = load("phiT", [128, NTAR])
        wqat_s = load("WqAT", [128, 128])
        wqpt_s = load("WqPT", [128, 128])
        wkt_s = load("WkT", [128, 128])
        wvt_s = load("WvT", [128, 128])
        w1t_s = load("W1T", [128, 128])
        bq_s = load("bq_c", [128, 1])
        bk_s = load("bk_c", [128, 1])
        bv_s = load("bv_rep", [128, 128])
        b1_s = load("b1_rep", [128, 128])
        w2_s = load("W2_rep", [128, 128])
        if ln_general:
            g_s = load("g_rep", [128, 128])
            lb_s = load("lb_rep", [128, 128])
        b2_s = singles.tile([128, 1], dt, tag="b2")
        b2ap = T["b2_t"][:]
        nc.sync.dma_start(
            out=b2_s[:], in_=_bc(b2ap, [[0, 128], [1, 1]])
        )
        ident = singles.tile([128, 128], dt, tag="ident")
        make_identity(nc, ident[:])
        eps_s = singles.tile([128, 1], dt, tag="eps")
        nc.vector.memset(eps_s[:], LN_EPS)

        # ---- Qphi^T = WqP @ phi^T + bq : [j=128, t=256] ----
        qphi_ps = ps_b.tile([128, NTAR], dt, tag="ps")
        nc.tensor.matmul(qphi_ps[:], wqpt_s[:], phiT_s[:], start=True, stop=True)
        qphiT = singles.tile([128, NTAR], dt, tag="qphiT")
        nc.scalar.add(out=qphiT[:], in_=qphi_ps[:], add=bq_s[:])

        # ---- QA^T : [j=128, (b,h)=24] ----
        qa_ps = ps_b.tile([128, BPC * H], dt, tag="ps")
        nc.tensor.matmul(qa_ps[:], wqat_s[:], at_s[:], start=True, stop=True)
        qaT = singles.tile([128, BPC * H], dt, tag="qaT")
        nc.scalar.copy(out=qaT[:], in_=qa_ps[:])

        # ---- block-diagonal (head-masked) Qphi / QA so the per-head K=32
        # score contractions become full-K=128 matmuls (avoids row-packed
        # tile_position matmuls, which fault on concurrent same-bank psum
        # writes here) ----
        qblk = singles.tile([128, NH, NTAR], dt, tag="qblk")
        nc.vector.memset(qblk[:], 0.0)
        qablk = singles.tile([128, NH, BPC * H], dt, tag="qablk")
        nc.vector.memset(qablk[:], 0.0)
        for n in range(NH):
            nc.vector.tensor_copy(
                out=qblk[32 * n : 32 * (n + 1), n, :],
                in_=qphiT[32 * n : 32 * (n + 1), :],
            )
            nc.vector.tensor_copy(
                out=qablk[32 * n : 32 * (n + 1), n, :],
                in_=qaT[32 * n : 32 * (n + 1), :],
            )

        y_all = singles.tile([128, BPC, H, TC], dt, tag="y_all")

        for b in range(BPC):
            # ---- H^T via PE transpose: [c=128, o=512] ----
            htb = sb_ht.tile([128, NOBS], dt, tag="htb")
            for oc in range(OC):
                hchunk = sb_ht.tile([128, 128], dt, tag="hchunk")
                nc.sync.dma_start(
                    out=hchunk[:], in_=T["Hemb"][b, oc * 128 : (oc + 1) * 128, :]
                )
                ht_ps = ps_b.tile([128, 128], dt, tag="ps")
                nc.tensor.transpose(ht_ps[:], hchunk[:], ident[:])
                nc.scalar.copy(
                    out=htb[:, oc * 128 : (oc + 1) * 128], in_=ht_ps[:]
                )

            # ---- K^T = Wk @ H^T + bk : [j=128, o=512] ----
            kt_ps = ps_b.tile([128, NOBS], dt, tag="ps")
            nc.tensor.matmul(kt_ps[:], wkt_s[:], htb[:], start=True, stop=True)
            ktb = sb_kt.tile([128, NOBS], dt, tag="ktb")
            nc.scalar.add(out=ktb[:], in_=kt_ps[:], add=bk_s[:])

            # ---- per o-chunk: V, SA->w, Sphi->U, V' ----
            vp_tiles = []
            ut_tiles = []
            for oc in range(OC):
                # V chunk [o=128, j=128]
                v_ps = ps_b.tile([128, 128], dt, tag="ps")
                nc.tensor.matmul(
                    v_ps[:],
                    htb[:, oc * 128 : (oc + 1) * 128],
                    wvt_s[:],
                    start=True,
                    stop=True,
                )
                # V1a [o, n, d33]: (V + bv | 1)
                v1a = sb_v1a.tile([128, NH, DH + 1], dt, tag="v1a")
                nc.vector.tensor_add(
                    out=v1a[:, :, 0:DH],
                    in0=v_ps[:].rearrange("p (n d) -> p n d", n=NH),
                    in1=bv_s[:].rearrange("p (n d) -> p n d", n=NH),
                )
                nc.vector.memset(v1a[:, :, DH : DH + 1], 1.0)

                # SA^T [o, (n,h)=48] : one K=128 matmul vs head-masked QA
                sa_ps = ps_b.tile([128, NH * H], dt, tag="ps")
                nc.tensor.matmul(
                    sa_ps[:],
                    ktb[:, oc * 128 : (oc + 1) * 128],
                    qablk[:, :, b * H : (b + 1) * H],
                    start=True,
                    stop=True,
                )
                w_oc = sb_w.tile([128, NH * H], dt, tag="w_oc")
                nc.scalar.activation(
                    out=w_oc[:], in_=sa_ps[:], func=AF.Exp, scale=SCALE
                )

                # Sphi^T (-> U) [o, n, t=256] : two K=128 N=512 matmuls
                u_ps = ps_u.tile([128, NH, NTAR], dt, tag="psu")
                for i in range(2):
                    nc.tensor.matmul(
                        u_ps[:, 2 * i : 2 * (i + 1), :],
                        ktb[:, oc * 128 : (oc + 1) * 128],
                        qblk[:, 2 * i : 2 * (i + 1), :],
                        start=True,
                        stop=True,
                    )
                ut = sb_ut.tile([128, NH, NTAR], dt, tag="ut")
                nc.scalar.activation(
                    out=ut[:], in_=u_ps[:], func=AF.Exp, scale=SCALE
                )
                ut_tiles.append(ut)

                # V' [o, n, h, d33] = V1a * w (broadcast h / d33)
                vp = sb_vp.tile([128, NH, H, DH + 1], dt, tag="vp")
                v1a_ap = v1a[:]
                in0 = _bc(
                    v1a_ap,
                    [v1a_ap.ap[0], [DH + 1, NH], [0, H], [1, DH + 1]],
                )
                w_ap = w_oc[:]
                in1 = _bc(
                    w_ap, [w_ap.ap[0], [H, NH], [1, H], [0, DH + 1]]
                )
                nc.vector.tensor_mul(out=vp[:], in0=in0, in1=in1)
                vp_tiles.append(vp)

            # ---- ctx (numerator | denominator) + normalize -> x1 ----
            for tcc in range(TC):
                x1 = sb_x1.tile([128, H, NH, DH], dt, tag="x1")
                for n in range(NH):
                    ctx_ps = ps_b.tile([128, H, DH + 1], dt, tag="ps")
                    for oc in range(OC):
                        nc.tensor.matmul(
                            ctx_ps[:],
                            ut_tiles[oc][:, n, tcc * 128 : (tcc + 1) * 128],
                            vp_tiles[oc][:, n, :, :],
                            start=(oc == 0),
                            stop=(oc == OC - 1),
                        )
                    rec = sb_stats.tile([128, H, 1], dt, tag="rec")
                    nc.vector.reciprocal(
                        out=rec[:], in_=ctx_ps[:, :, DH : DH + 1]
                    )
                    rec_ap = rec[:]
                    rec_bc = _bc(rec_ap, [rec_ap.ap[0], [1, H], [0, DH]])
                    nc.vector.tensor_mul(
                        out=x1[:, :, n, :], in0=ctx_ps[:, :, 0:DH], in1=rec_bc
                    )

                # ---- transpose x1 per h -> x1T [j=128, t=128]; FFN ----
                mv_all = sb_stats.tile([128, H, 2], dt, tag="mv")
                x2s_tiles = []
                for h in range(H):
                    x1t_ps = ps_b.tile([128, 128], dt, tag="ps")
                    nc.tensor.transpose(x1t_ps[:], x1[:, h, :, :], ident[:])
                    x1t = sb_x1t.tile([128, 128], dt, tag="x1t")
                    nc.scalar.copy(out=x1t[:], in_=x1t_ps[:])

                    x2_ps = ps_b.tile([128, 128], dt, tag="ps")
                    nc.tensor.matmul(
                        x2_ps[:], x1t[:], w1t_s[:], start=True, stop=True
                    )
                    # evac + b1
                    x2s = sb_x2s.tile([128, 128], dt, tag="x2s")
                    nc.vector.tensor_add(out=x2s[:], in0=x2_ps[:], in1=b1_s[:])
                    stats = sb_ffn.tile([128, 6], dt, tag="bnst")
                    nc.vector.bn_stats(out=stats[:], in_=x2s[:])
                    nc.vector.bn_aggr(out=mv_all[:, h, :], in_=stats[:])
                    x2s_tiles.append(x2s)

                # batched LN stats for the 12 h tiles
                std = sb_stats.tile([128, H], dt, tag="std")
                nc.scalar.activation(
                    out=std[:], in_=mv_all[:, :, 1], func=AF.Sqrt, bias=eps_s[:]
                )
                rstd = sb_stats.tile([128, H], dt, tag="rstd")
                nc.vector.reciprocal(out=rstd[:], in_=std[:])
                nmr = sb_stats.tile([128, H], dt, tag="nmr")
                nc.vector.tensor_mul(
                    out=nmr[:], in0=mv_all[:, :, 0], in1=rstd[:]
                )
                nc.vector.tensor_scalar_mul(out=nmr[:], in0=nmr[:], scalar1=-1.0)

                for h in range(H):
                    x2s = x2s_tiles[h]
                    if not ln_general:
                        x3 = sb_ffn.tile([128, 128], dt, tag="x3")
                        nc.scalar.activation(
                            out=x3[:],
                            in_=x2s[:],
                            func=AF.Relu,
                            scale=rstd[:, h : h + 1],
                            bias=nmr[:, h : h + 1],
                        )
                    else:
                        xn = sb_ffn.tile([128, 128], dt, tag="xn")
                        nc.scalar.activation(
                            out=xn[:],
                            in_=x2s[:],
                            func=AF.Identity,
                            scale=rstd[:, h : h + 1],
                            bias=nmr[:, h : h + 1],
                        )
                        nc.vector.tensor_mul(out=xn[:], in0=xn[:], in1=g_s[:])
                        nc.vector.tensor_add(out=xn[:], in0=xn[:], in1=lb_s[:])
                        x3 = sb_ffn.tile([128, 128], dt, tag="x3")
                        nc.scalar.activation(
                            out=x3[:], in_=xn[:], func=AF.Relu
                        )
                    scr = sb_ffn.tile([128, 128], dt, tag="scr")
                    nc.gpsimd.tensor_mul(out=scr[:], in0=x3[:], in1=w2_s[:])
                    nc.vector.tensor_reduce(
                        out=y_all[:, b, h, tcc : tcc + 1],
                        in_=scr[:],
                        axis=mybir.AxisListType.X,
                        op=ALU.add,
                    )

        # ---- finalize y: +b2, transpose, DMA out ----
        nc.vector.tensor_scalar_add(out=y_all[:], in0=y_all[:], scalar1=b2_s[:])
        y_ps = ps_b.tile([BPC * H * TC, 128], dt, tag="ps")
        nc.tensor.transpose(y_ps[:], y_all[:], ident[:])
        yT = singles.tile([BPC * H * TC, 128], dt, tag="yT")
        nc.scalar.copy(out=yT[:], in_=y_ps[:])
        nc.sync.dma_start(out=T["Y"][:], in_=yT[:])


# ------------------------------------------------------------------
def prepare_in_maps(inputs):
    A = np.ascontiguousarray(inputs["A"], np.float32)
    phi = np.ascontiguousarray(inputs["phi_tar"], np.float32)
    Hm = np.ascontiguousarray(inputs["H_emb_obs"], np.float32)
    Wq = np.asarray(inputs["Wq"], np.float32)
    Wk = np.asarray(inputs["Wk"], np.float32)
    Wv = np.asarray(inputs["Wv"], np.float32)
    W1 = np.asarray(inputs["W1"], np.float32)
    W2 = np.asarray(inputs["W2"], np.float32)

    shared = {
        "phiT": np.ascontiguousarray(phi.T),
        "WqAT": np.ascontiguousarray(Wq[:, :DTOT].T),
        "WqPT": np.ascontiguousarray(Wq[:, DTOT:].T),
        "WkT": np.ascontiguousarray(Wk.T),
        "WvT": np.ascontiguousarray(Wv.T),
        "W1T": np.ascontiguousarray(W1.T),
        "bq_c": np.ascontiguousarray(
            np.asarray(inputs["bq"], np.float32).reshape(128, 1)
        ),
        "bk_c": np.ascontiguousarray(
            np.asarray(inputs["bk"], np.float32).reshape(128, 1)
        ),
        "bv_rep": np.ascontiguousarray(
            np.broadcast_to(np.asarray(inputs["bv"], np.float32), (128, 128))
        ),
        "b1_rep": np.ascontiguousarray(
            np.broadcast_to(np.asarray(inputs["b1"], np.float32), (128, 128))
        ),
        "W2_rep": np.ascontiguousarray(np.broadcast_to(W2[0], (128, 128))),
        "b2_t": np.ascontiguousarray(
            np.asarray(inputs["b2"], np.float32).reshape(1, 1)
        ),
    }
    ln_g = np.asarray(inputs["ln_g"], np.float32)
    ln_b = np.asarray(inputs["ln_b"], np.float32)
    ln_general = not (
        np.allclose(ln_g, 1.0, atol=1e-7) and np.allclose(ln_b, 0.0, atol=1e-7)
    )
    if ln_general:
        shared["g_rep"] = np.ascontiguousarray(np.broadcast_to(ln_g, (128, 128)))
        shared["lb_rep"] = np.ascontiguousarray(np.broadcast_to(ln_b, (128, 128)))

    in_maps = []
    for k in range(NCORES):
        sl = slice(k * BPC, (k + 1) * BPC)
        m = dict(shared)
        m["AT"] = np.ascontiguousarray(
            A[sl].transpose(2, 0, 1).reshape(128, BPC * H)
        )
        m["Hemb"] = np.ascontiguousarray(Hm[sl])
        in_maps.append(m)
    return in_maps, ln_general


_nc_cache = {}
last_exec_time_ns = None
last_trace_path = None


def get_nc(ln_general: bool):
    if ln_general not in _nc_cache:
        _nc_cache[ln_general] = build_nc(ln_general)
    return _nc_cache[ln_general]


def kernel(**inputs) -> np.ndarray:
    global last_exec_time_ns, last_trace_path
    in_maps, ln_general = prepare_in_maps(inputs)
    nc = get_nc(ln_general)
    trace = bool(int(os.environ.get("KERNEL_TRACE", "0")))
    res = run_bass_kernel_spmd(
        nc, in_maps, core_ids=list(range(NCORES)), trace=trace
    )
    last_exec_time_ns = res.exec_time_ns
    if res.instructions_and_trace is not None:
        last_trace_path = res.instructions_and_trace[1]
    out = np.empty((B, H, NTAR, 1), np.float32)
    for k in range(NCORES):
        yk = res.results[k]["Y"].reshape(BPC, H, TC * 128)
        out[k * BPC : (k + 1) * BPC, :, :, 0] = yk
    return out



# revision 17
# speedup vs baseline: 1.4353x; 1.4353x over previous
"""Trainium2 Bass kernel for nn_CrossAttnHead (cross-attention head + FFN).

Math (reference):
  Q = concat(A bcast over t, phi_tar bcast over (b,h)) @ Wq^T + bq
  K,V = H_emb_obs @ {Wk,Wv}^T + b
  scores = (Qh . Kh)/sqrt(dh) ; attn = softmax(scores, axis=o)
  ctx = attn @ Vh ; y = Linear2(relu(LN(Linear1(ctx))))

Key structure exploited on device:
  Q[b,h,t] = QA[b,h] + Qphi[t]  (concat-linear splits into two small matmuls)
  => scores[b,h,n,t,o] = SA[b,h,n,o] + Sphi[b,n,t,o]
  => exp(scores/s) = w[b,h,n,o] * U[b,n,t,o],  w = exp(SA/s), U = exp(Sphi/s)
  so softmax numerator/denominator come from one matmul of U against
  w-scaled V (with a ones-column appended for the denominator).

Sharding: data-parallel over B: 16 batches -> 8 cores x 2 batches.
"""

import os
import numpy as np

import concourse.bass as bass
import concourse.mybir as mybir
import concourse.tile as tile
from concourse.bass_utils import run_bass_kernel_spmd
from concourse.masks import make_identity
from concourse.vector_clock import ScopedClock
import bass_rust

# ---- problem constants (hardcoded per contract) ----
B, H, NTAR, NOBS = 16, 12, 256, 512
DSITE = 128          # d (site embedding)
DTOT = 128           # d_tot == h_temporal
M_BASIS = 128        # phi basis dim
NH, DH = 4, 32       # heads
NCORES = 8
BPC = B // NCORES    # batches per core = 2
SCALE = 1.0 / np.sqrt(DH)
LN_EPS = 1e-5

f32 = mybir.dt.float32
bf16 = mybir.dt.bfloat16
AF = mybir.ActivationFunctionType
ALU = mybir.AluOpType

OC = 4               # NOBS / 128 chunks
TC = 2               # NTAR / 128 chunks


# ------------------------------------------------------------------
# walrus in this container rejects >1 sem wait per instruction (CTRL Drain,
# matmul LDWEIGHTS struct, ...). Two patches:
#  1) _add_instruction: hoist all-but-one waits of any instruction onto
#     same-engine NoOps inserted just before it (sequencer semantics are
#     identical: all waits must pass before the inst dispatches).
#  2) _drain_and_barrier: the tail drain gets its waits added after
#     insertion, so split it into one drain per wait.
def _install_drain_patch():
    _orig_add = tile.TileContext._add_instruction

    def _add_split(self, inst):
        si = getattr(inst, "sync_info", None)
        if si is not None and si.on_wait and len(si.on_wait) > 1:
            waits = list(si.on_wait)
            si.on_wait = waits[-1:]
            for w in waits[:-1]:
                nop = mybir.InstNoOp(
                    name=self.nc.get_next_instruction_name(),
                    sync_info=mybir.SyncInfo(on_wait=[w], on_update=[]),
                    bass_nofuse=True,
                    engine=inst.engine,
                )
                _orig_add(self, nop)
        _orig_add(self, inst)

    tile.TileContext._add_instruction = _add_split

    def _patched(self, tick_clock, wait_clock):
        d0 = self.nc.sync.drain()
        wait_clock.add_sem_waits(
            d0.ins, ScopedClock({None: tick_clock.global_clock})
        )
        si = d0.ins.sync_info
        if si is not None and si.on_wait and len(si.on_wait) > 1:
            waits = list(si.on_wait)
            si.on_wait = waits[:1]
            for w in waits[1:]:
                d = self.nc.sync.drain()
                dsi = d.ins.sync_info
                if dsi is None:
                    d.ins.sync_info = bass_rust.SyncInfo(
                        on_wait=[w], on_update=[]
                    )
                else:
                    dsi.on_wait = [w]
        self.nc.all_engine_barrier()
        popped = self.nc._tile_sem_poison_stack.pop()
        assert popped is self._sem_poison
        # chunk the sem range-clears: wide EVENT_SEMAPHORE_RANGE_CLEAR
        # trips "ISA wrong length" in this walrus build
        sems = list(self.sems.allocated().values())
        for i in range(0, len(sems), 4):
            self.nc.clear_and_free_semaphores(sems[i : i + 4])
        self.nc.all_engine_barrier()

    tile.TileContext._drain_and_barrier = _patched


_install_drain_patch()


def _bc(ap, ap_dims):
    """AP with the same tensor/offset but explicit [step, count] dims."""
    return bass.AP(tensor=ap.tensor, offset=ap.offset, ap=ap_dims)


def build_nc(ln_general: bool):
    nc = bass.Bass()
    dt = f32

    # ---- DRAM I/O ----
    AT = nc.dram_tensor("AT", [128, BPC * H], bf16, kind="ExternalInput")
    Hemb = nc.dram_tensor("Hemb", [BPC, NOBS, DSITE], bf16, kind="ExternalInput")
    phiT = nc.dram_tensor("phiT", [128, NTAR], bf16, kind="ExternalInput")
    WqAT = nc.dram_tensor("WqAT", [128, 128], bf16, kind="ExternalInput")
    WqPT = nc.dram_tensor("WqPT", [128, 128], bf16, kind="ExternalInput")
    WkT = nc.dram_tensor("WkT", [128, 128], bf16, kind="ExternalInput")
    WvT = nc.dram_tensor("WvT", [128, 128], bf16, kind="ExternalInput")
    W1T = nc.dram_tensor("W1T", [128, 128], bf16, kind="ExternalInput")
    bq_c = nc.dram_tensor("bq_c", [128, 1], dt, kind="ExternalInput")
    bk_c = nc.dram_tensor("bk_c", [128, 1], dt, kind="ExternalInput")
    bv_rep = nc.dram_tensor("bv_rep", [128, 128], dt, kind="ExternalInput")
    b1_rep = nc.dram_tensor("b1_rep", [128, 128], dt, kind="ExternalInput")
    W2_rep = nc.dram_tensor("W2_rep", [128, 128], bf16, kind="ExternalInput")
    b2_t = nc.dram_tensor("b2_t", [1, 1], dt, kind="ExternalInput")
    if ln_general:
        g_rep_t = nc.dram_tensor("g_rep", [128, 128], dt, kind="ExternalInput")
        lb_rep_t = nc.dram_tensor("lb_rep", [128, 128], dt, kind="ExternalInput")
    Y = nc.dram_tensor("Y", [BPC * H * TC, 128], dt, kind="ExternalOutput")

    with tile.TileContext(nc) as tc:
        _emit(nc, tc, locals(), ln_general)
    return nc


def _emit(nc, tc, T, ln_general):
    dt = f32
    from contextlib import ExitStack

    ctxmgr = ExitStack()
    with ctxmgr:
        singles = ctxmgr.enter_context(tc.tile_pool(name="singles", bufs=1))
        sb_ht = ctxmgr.enter_context(tc.tile_pool(name="ht", bufs=2))
        sb_kt = ctxmgr.enter_context(tc.tile_pool(name="kt", bufs=2))
        sb_v1a = ctxmgr.enter_context(tc.tile_pool(name="v1a", bufs=3))
        sb_w = ctxmgr.enter_context(tc.tile_pool(name="wp", bufs=3))
        sb_vp = ctxmgr.enter_context(tc.tile_pool(name="vp", bufs=5))
        sb_ut = ctxmgr.enter_context(tc.tile_pool(name="ut", bufs=5))
        sb_x1 = ctxmgr.enter_context(tc.tile_pool(name="x1", bufs=3))
        sb_x1t = ctxmgr.enter_context(tc.tile_pool(name="x1t", bufs=3))
        sb_ffn = ctxmgr.enter_context(tc.tile_pool(name="ffn", bufs=3))
        # all 12 h-tiles of a (b,tc) group stay live until the batched LN
        # stats complete -> needs >=12 slots plus pipelining slack
        sb_x2s = ctxmgr.enter_context(tc.tile_pool(name="x2s", bufs=14))
        sb_stats = ctxmgr.enter_context(tc.tile_pool(name="stats", bufs=3))
        ps_u = ctxmgr.enter_context(tc.tile_pool(name="psu", bufs=2, space="PSUM"))
        ps_b = ctxmgr.enter_context(tc.tile_pool(name="psb", bufs=4, space="PSUM"))

        # ---- load constants ----
        def load(name, shape, tdt=dt):
            t = singles.tile(shape, tdt, tag=name)
            nc.sync.dma_start(out=t[:], in_=T[name][:])
            return t

        at_s = load("AT", [128, BPC * H], bf16)
        phiT_s = load("phiT", [128, NTAR], bf16)
        wqat_s = load("WqAT", [128, 128], bf16)
        wqpt_s = load("WqPT", [128, 128], bf16)
        wkt_s = load("WkT", [128, 128], bf16)
        wvt_s = load("WvT", [128, 128], bf16)
        w1t_s = load("W1T", [128, 128], bf16)
        bq_s = load("bq_c", [128, 1])
        bk_s = load("bk_c", [128, 1])
        bv_s = load("bv_rep", [128, 128])
        b1_s = load("b1_rep", [128, 128])
        w2_s = load("W2_rep", [128, 128], bf16)
        if ln_general:
            g_s = load("g_rep", [128, 128])
            lb_s = load("lb_rep", [128, 128])
        b2_s = singles.tile([128, 1], dt, tag="b2")
        b2ap = T["b2_t"][:]
        nc.sync.dma_start(
            out=b2_s[:], in_=_bc(b2ap, [[0, 128], [1, 1]])
        )
        ident = singles.tile([128, 128], bf16, tag="ident")
        make_identity(nc, ident[:])
        identf = singles.tile([128, 128], dt, tag="identf")
        make_identity(nc, identf[:])
        eps_s = singles.tile([128, 1], dt, tag="eps")
        nc.vector.memset(eps_s[:], LN_EPS)

        # ---- Qphi^T = WqP @ phi^T + bq : [j=128, t=256] ----
        qphi_ps = ps_b.tile([128, NTAR], dt, tag="ps")
        nc.tensor.matmul(qphi_ps[:], wqpt_s[:], phiT_s[:], start=True, stop=True)
        qphiT = singles.tile([128, NTAR], bf16, tag="qphiT")
        nc.scalar.add(out=qphiT[:], in_=qphi_ps[:], add=bq_s[:])

        # ---- QA^T : [j=128, (b,h)=24] ----
        qa_ps = ps_b.tile([128, BPC * H], dt, tag="ps")
        nc.tensor.matmul(qa_ps[:], wqat_s[:], at_s[:], start=True, stop=True)
        qaT = singles.tile([128, BPC * H], bf16, tag="qaT")
        nc.scalar.copy(out=qaT[:], in_=qa_ps[:])

        # ---- block-diagonal (head-masked) Qphi / QA so the per-head K=32
        # score contractions become full-K=128 matmuls (avoids row-packed
        # tile_position matmuls, which fault on concurrent same-bank psum
        # writes here) ----
        qblk = singles.tile([128, NH, NTAR], bf16, tag="qblk")
        nc.vector.memset(qblk[:], 0.0)
        qablk = singles.tile([128, NH, BPC * H], bf16, tag="qablk")
        nc.vector.memset(qablk[:], 0.0)
        for n in range(NH):
            nc.vector.tensor_copy(
                out=qblk[32 * n : 32 * (n + 1), n, :],
                in_=qphiT[32 * n : 32 * (n + 1), :],
            )
            nc.vector.tensor_copy(
                out=qablk[32 * n : 32 * (n + 1), n, :],
                in_=qaT[32 * n : 32 * (n + 1), :],
            )

        y_all = singles.tile([128, BPC, H, TC], dt, tag="y_all")

        for b in range(BPC):
            # ---- H^T via PE transpose: [c=128, o=512] ----
            htb = sb_ht.tile([128, NOBS], bf16, tag="htb")
            for oc in range(OC):
                hchunk = sb_ht.tile([128, 128], bf16, tag="hchunk")
                nc.sync.dma_start(
                    out=hchunk[:], in_=T["Hemb"][b, oc * 128 : (oc + 1) * 128, :]
                )
                ht_ps = ps_b.tile([128, 128], bf16, tag="ps")
                nc.tensor.transpose(ht_ps[:], hchunk[:], ident[:])
                nc.scalar.copy(
                    out=htb[:, oc * 128 : (oc + 1) * 128], in_=ht_ps[:]
                )

            # ---- K^T = Wk @ H^T + bk : [j=128, o=512] ----
            kt_ps = ps_b.tile([128, NOBS], dt, tag="ps")
            nc.tensor.matmul(kt_ps[:], wkt_s[:], htb[:], start=True, stop=True)
            ktb = sb_kt.tile([128, NOBS], bf16, tag="ktb")
            nc.scalar.add(out=ktb[:], in_=kt_ps[:], add=bk_s[:])

            # ---- per o-chunk: V, SA->w, Sphi->U, V' ----
            vp_tiles = []
            ut_tiles = []
            for oc in range(OC):
                # V chunk [o=128, j=128]
                v_ps = ps_b.tile([128, 128], dt, tag="ps")
                nc.tensor.matmul(
                    v_ps[:],
                    htb[:, oc * 128 : (oc + 1) * 128],
                    wvt_s[:],
                    start=True,
                    stop=True,
                )
                # V1a [o, n, d33]: (V + bv | 1)
                v1a = sb_v1a.tile([128, NH, DH + 1], bf16, tag="v1a")
                nc.vector.tensor_add(
                    out=v1a[:, :, 0:DH],
                    in0=v_ps[:].rearrange("p (n d) -> p n d", n=NH),
                    in1=bv_s[:].rearrange("p (n d) -> p n d", n=NH),
                )
                nc.vector.memset(v1a[:, :, DH : DH + 1], 1.0)

                # SA^T [o, (n,h)=48] : one K=128 matmul vs head-masked QA
                sa_ps = ps_b.tile([128, NH * H], dt, tag="ps")
                nc.tensor.matmul(
                    sa_ps[:],
                    ktb[:, oc * 128 : (oc + 1) * 128],
                    qablk[:, :, b * H : (b + 1) * H],
                    start=True,
                    stop=True,
                )
                w_oc = sb_w.tile([128, NH * H], bf16, tag="w_oc")
                nc.scalar.activation(
                    out=w_oc[:], in_=sa_ps[:], func=AF.Exp, scale=SCALE
                )

                # Sphi^T (-> U) [o, n, t=256] : two K=128 N=512 matmuls
                u_ps = ps_u.tile([128, NH, NTAR], dt, tag="psu")
                for i in range(2):
                    nc.tensor.matmul(
                        u_ps[:, 2 * i : 2 * (i + 1), :],
                        ktb[:, oc * 128 : (oc + 1) * 128],
                        qblk[:, 2 * i : 2 * (i + 1), :],
                        start=True,
                        stop=True,
                    )
                ut = sb_ut.tile([128, NH, NTAR], bf16, tag="ut")
                nc.scalar.activation(
                    out=ut[:], in_=u_ps[:], func=AF.Exp, scale=SCALE
                )
                ut_tiles.append(ut)

                # V' [o, n, h, d33] = V1a * w (broadcast h / d33)
                vp = sb_vp.tile([128, NH, H, DH + 1], bf16, tag="vp")
                v1a_ap = v1a[:]
                in0 = _bc(
                    v1a_ap,
                    [v1a_ap.ap[0], [DH + 1, NH], [0, H], [1, DH + 1]],
                )
                w_ap = w_oc[:]
                in1 = _bc(
                    w_ap, [w_ap.ap[0], [H, NH], [1, H], [0, DH + 1]]
                )
                nc.vector.tensor_mul(out=vp[:], in0=in0, in1=in1)
                vp_tiles.append(vp)

            # ---- ctx (numerator | denominator) + normalize -> x1 ----
            for tcc in range(TC):
                x1 = sb_x1.tile([128, H, NH, DH], bf16, tag="x1")
                for n in range(NH):
                    ctx_ps = ps_b.tile([128, H, DH + 1], dt, tag="ps")
                    for oc in range(OC):
                        nc.tensor.matmul(
                            ctx_ps[:],
                            ut_tiles[oc][:, n, tcc * 128 : (tcc + 1) * 128],
                            vp_tiles[oc][:, n, :, :],
                            start=(oc == 0),
                            stop=(oc == OC - 1),
                        )
                    rec = sb_stats.tile([128, H, 1], dt, tag="rec")
                    nc.vector.reciprocal(
                        out=rec[:], in_=ctx_ps[:, :, DH : DH + 1]
                    )
                    rec_ap = rec[:]
                    rec_bc = _bc(rec_ap, [rec_ap.ap[0], [1, H], [0, DH]])
                    nc.vector.tensor_mul(
                        out=x1[:, :, n, :], in0=ctx_ps[:, :, 0:DH], in1=rec_bc
                    )

                # ---- transpose x1 per h -> x1T [j=128, t=128]; FFN ----
                mv_all = sb_stats.tile([128, H, 2], dt, tag="mv")
                x2s_tiles = []
                for h in range(H):
                    x1t_ps = ps_b.tile([128, 128], bf16, tag="ps")
                    nc.tensor.transpose(x1t_ps[:], x1[:, h, :, :], ident[:])
                    x1t = sb_x1t.tile([128, 128], bf16, tag="x1t")
                    nc.scalar.copy(out=x1t[:], in_=x1t_ps[:])

                    x2_ps = ps_b.tile([128, 128], dt, tag="ps")
                    nc.tensor.matmul(
                        x2_ps[:], x1t[:], w1t_s[:], start=True, stop=True
                    )
                    # evac + b1
                    x2s = sb_x2s.tile([128, 128], bf16, tag="x2s")
                    nc.vector.tensor_add(out=x2s[:], in0=x2_ps[:], in1=b1_s[:])
                    stats = sb_ffn.tile([128, 6], dt, tag="bnst")
                    nc.vector.bn_stats(out=stats[:], in_=x2s[:])
                    nc.vector.bn_aggr(out=mv_all[:, h, :], in_=stats[:])
                    x2s_tiles.append(x2s)

                # batched LN stats for the 12 h tiles
                std = sb_stats.tile([128, H], dt, tag="std")
                nc.scalar.activation(
                    out=std[:], in_=mv_all[:, :, 1], func=AF.Sqrt, bias=eps_s[:]
                )
                rstd = sb_stats.tile([128, H], dt, tag="rstd")
                nc.vector.reciprocal(out=rstd[:], in_=std[:])
                nmr = sb_stats.tile([128, H], dt, tag="nmr")
                nc.vector.tensor_mul(
                    out=nmr[:], in0=mv_all[:, :, 0], in1=rstd[:]
                )
                nc.vector.tensor_scalar_mul(out=nmr[:], in0=nmr[:], scalar1=-1.0)

                for h in range(H):
                    x2s = x2s_tiles[h]
                    if not ln_general:
                        x3 = sb_ffn.tile([128, 128], bf16, tag="x3")
                        nc.scalar.activation(
                            out=x3[:],
                            in_=x2s[:],
                            func=AF.Relu,
                            scale=rstd[:, h : h + 1],
                            bias=nmr[:, h : h + 1],
                        )
                    else:
                        xn = sb_ffn.tile([128, 128], dt, tag="xn")
                        nc.scalar.activation(
                            out=xn[:],
                            in_=x2s[:],
                            func=AF.Identity,
                            scale=rstd[:, h : h + 1],
                            bias=nmr[:, h : h + 1],
                        )
                        nc.vector.tensor_mul(out=xn[:], in0=xn[:], in1=g_s[:])
                        nc.vector.tensor_add(out=xn[:], in0=xn[:], in1=lb_s[:])
                        x3 = sb_ffn.tile([128, 128], bf16, tag="x3")
                        nc.scalar.activation(
                            out=x3[:], in_=xn[:], func=AF.Relu
                        )
                    scr = sb_ffn.tile([128, 128], bf16, tag="scr")
                    nc.gpsimd.tensor_mul(out=scr[:], in0=x3[:], in1=w2_s[:])
                    nc.vector.tensor_reduce(
                        out=y_all[:, b, h, tcc : tcc + 1],
                        in_=scr[:],
                        axis=mybir.AxisListType.X,
                        op=ALU.add,
                    )

        # ---- finalize y: +b2, transpose, DMA out ----
        nc.vector.tensor_scalar_add(out=y_all[:], in0=y_all[:], scalar1=b2_s[:])
        y_ps = ps_b.tile([BPC * H * TC, 128], dt, tag="ps")
        nc.tensor.transpose(y_ps[:], y_all[:], identf[:])
        yT = singles.tile([BPC * H * TC, 128], dt, tag="yT")
        nc.scalar.copy(out=yT[:], in_=y_ps[:])
        nc.sync.dma_start(out=T["Y"][:], in_=yT[:])


# ------------------------------------------------------------------
def prepare_in_maps(inputs):
    import ml_dtypes

    bf = ml_dtypes.bfloat16
    A = np.ascontiguousarray(inputs["A"], np.float32)
    phi = np.ascontiguousarray(inputs["phi_tar"], np.float32)
    Hm = np.ascontiguousarray(inputs["H_emb_obs"], np.float32)
    Wq = np.asarray(inputs["Wq"], np.float32)
    Wk = np.asarray(inputs["Wk"], np.float32)
    Wv = np.asarray(inputs["Wv"], np.float32)
    W1 = np.asarray(inputs["W1"], np.float32)
    W2 = np.asarray(inputs["W2"], np.float32)

    shared = {
        "phiT": np.ascontiguousarray(phi.T.astype(bf)),
        "WqAT": np.ascontiguousarray(Wq[:, :DTOT].T.astype(bf)),
        "WqPT": np.ascontiguousarray(Wq[:, DTOT:].T.astype(bf)),
        "WkT": np.ascontiguousarray(Wk.T.astype(bf)),
        "WvT": np.ascontiguousarray(Wv.T.astype(bf)),
        "W1T": np.ascontiguousarray(W1.T.astype(bf)),
        "bq_c": np.ascontiguousarray(
            np.asarray(inputs["bq"], np.float32).reshape(128, 1)
        ),
        "bk_c": np.ascontiguousarray(
            np.asarray(inputs["bk"], np.float32).reshape(128, 1)
        ),
        "bv_rep": np.ascontiguousarray(
            np.broadcast_to(np.asarray(inputs["bv"], np.float32), (128, 128))
        ),
        "b1_rep": np.ascontiguousarray(
            np.broadcast_to(np.asarray(inputs["b1"], np.float32), (128, 128))
        ),
        "W2_rep": np.ascontiguousarray(
            np.broadcast_to(W2[0], (128, 128)).astype(bf)
        ),
        "b2_t": np.ascontiguousarray(
            np.asarray(inputs["b2"], np.float32).reshape(1, 1)
        ),
    }
    ln_g = np.asarray(inputs["ln_g"], np.float32)
    ln_b = np.asarray(inputs["ln_b"], np.float32)
    ln_general = not (
        np.allclose(ln_g, 1.0, atol=1e-7) and np.allclose(ln_b, 0.0, atol=1e-7)
    )
    if ln_general:
        shared["g_rep"] = np.ascontiguousarray(np.broadcast_to(ln_g, (128, 128)))
        shared["lb_rep"] = np.ascontiguousarray(np.broadcast_to(ln_b, (128, 128)))

    in_maps = []
    for k in range(NCORES):
        sl = slice(k * BPC, (k + 1) * BPC)
        m = dict(shared)
        m["AT"] = np.ascontiguousarray(
            A[sl].transpose(2, 0, 1).reshape(128, BPC * H).astype(bf)
        )
        m["Hemb"] = np.ascontiguousarray(Hm[sl].astype(bf))
        in_maps.append(m)
    return in_maps, ln_general


_nc_cache = {}
last_exec_time_ns = None
last_trace_path = None


def get_nc(ln_general: bool):
    if ln_general not in _nc_cache:
        _nc_cache[ln_general] = build_nc(ln_general)
    return _nc_cache[ln_general]


def kernel(**inputs) -> np.ndarray:
    global last_exec_time_ns, last_trace_path
    in_maps, ln_general = prepare_in_maps(inputs)
    nc = get_nc(ln_general)
    trace = bool(int(os.environ.get("KERNEL_TRACE", "0")))
    res = run_bass_kernel_spmd(
        nc, in_maps, core_ids=list(range(NCORES)), trace=trace
    )
    last_exec_time_ns = res.exec_time_ns
    if res.instructions_and_trace is not None:
        last_trace_path = res.instructions_and_trace[1]
    out = np.empty((B, H, NTAR, 1), np.float32)
    for k in range(NCORES):
        yk = res.results[k]["Y"].reshape(BPC, H, TC * 128)
        out[k * BPC : (k + 1) * BPC, :, :, 0] = yk
    return out



# revision 32
# speedup vs baseline: 1.5666x; 1.0915x over previous
"""Trainium2 Bass kernel for nn_CrossAttnHead (cross-attention head + FFN).

Math (reference):
  Q = concat(A bcast over t, phi_tar bcast over (b,h)) @ Wq^T + bq
  K,V = H_emb_obs @ {Wk,Wv}^T + b
  scores = (Qh . Kh)/sqrt(dh) ; attn = softmax(scores, axis=o)
  ctx = attn @ Vh ; y = Linear2(relu(LN(Linear1(ctx))))

Key structure exploited on device:
  Q[b,h,t] = QA[b,h] + Qphi[t]  (concat-linear splits into two small matmuls)
  => scores[b,h,n,t,o] = SA[b,h,n,o] + Sphi[b,n,t,o]
  => exp(scores/s) = w[b,h,n,o] * U[b,n,t,o],  w = exp(SA/s), U = exp(Sphi/s)
  so softmax numerator/denominator come from one matmul of U against
  w-scaled V (with a ones-column appended for the denominator).

Sharding: data-parallel over B: 16 batches -> 8 cores x 2 batches.
"""

import os
import numpy as np

import concourse.bass as bass
import concourse.mybir as mybir
import concourse.tile as tile
from concourse.bass_utils import run_bass_kernel_spmd
from concourse.masks import make_identity
from concourse.vector_clock import ScopedClock
import bass_rust

# ---- problem constants (hardcoded per contract) ----
B, H, NTAR, NOBS = 16, 12, 256, 512
DSITE = 128          # d (site embedding)
DTOT = 128           # d_tot == h_temporal
M_BASIS = 128        # phi basis dim
NH, DH = 4, 32       # heads
NCORES = 8
BPC = B // NCORES    # batches per core = 2
SCALE = 1.0 / np.sqrt(DH)
LN_EPS = 1e-5

f32 = mybir.dt.float32
bf16 = mybir.dt.bfloat16
AF = mybir.ActivationFunctionType
ALU = mybir.AluOpType

OC = 4               # NOBS / 128 chunks
TC = 2               # NTAR / 128 chunks


# ------------------------------------------------------------------
# walrus in this container rejects >1 sem wait per instruction (CTRL Drain,
# matmul LDWEIGHTS struct, ...). Two patches:
#  1) _add_instruction: hoist all-but-one waits of any instruction onto
#     same-engine NoOps inserted just before it (sequencer semantics are
#     identical: all waits must pass before the inst dispatches).
#  2) _drain_and_barrier: the tail drain gets its waits added after
#     insertion, so split it into one drain per wait.
def _install_drain_patch():
    _orig_add = tile.TileContext._add_instruction

    def _add_split(self, inst):
        si = getattr(inst, "sync_info", None)
        if si is not None and si.on_wait and len(si.on_wait) > 1:
            waits = list(si.on_wait)
            si.on_wait = waits[-1:]
            for w in waits[:-1]:
                nop = mybir.InstNoOp(
                    name=self.nc.get_next_instruction_name(),
                    sync_info=mybir.SyncInfo(on_wait=[w], on_update=[]),
                    bass_nofuse=True,
                    engine=inst.engine,
                )
                _orig_add(self, nop)
        _orig_add(self, inst)

    tile.TileContext._add_instruction = _add_split

    def _patched(self, tick_clock, wait_clock):
        d0 = self.nc.sync.drain()
        wait_clock.add_sem_waits(
            d0.ins, ScopedClock({None: tick_clock.global_clock})
        )
        si = d0.ins.sync_info
        if si is not None and si.on_wait and len(si.on_wait) > 1:
            waits = list(si.on_wait)
            si.on_wait = waits[:1]
            for w in waits[1:]:
                d = self.nc.sync.drain()
                dsi = d.ins.sync_info
                if dsi is None:
                    d.ins.sync_info = bass_rust.SyncInfo(
                        on_wait=[w], on_update=[]
                    )
                else:
                    dsi.on_wait = [w]
        self.nc.all_engine_barrier()
        popped = self.nc._tile_sem_poison_stack.pop()
        assert popped is self._sem_poison
        # chunk the sem range-clears: wide EVENT_SEMAPHORE_RANGE_CLEAR
        # trips "ISA wrong length" in this walrus build
        sems = list(self.sems.allocated().values())
        for i in range(0, len(sems), 4):
            self.nc.clear_and_free_semaphores(sems[i : i + 4])
        self.nc.all_engine_barrier()

    tile.TileContext._drain_and_barrier = _patched


_install_drain_patch()


def _bc(ap, ap_dims):
    """AP with the same tensor/offset but explicit [step, count] dims."""
    return bass.AP(tensor=ap.tensor, offset=ap.offset, ap=ap_dims)


def build_nc(ln_general: bool):
    nc = bass.Bass()
    dt = f32

    # ---- DRAM I/O ----
    AT = nc.dram_tensor("AT", [128, BPC * H], bf16, kind="ExternalInput")
    Hemb = nc.dram_tensor("Hemb", [BPC, NOBS, DSITE], bf16, kind="ExternalInput")
    phiT = nc.dram_tensor("phiT", [128, NTAR], bf16, kind="ExternalInput")
    WqAT = nc.dram_tensor("WqAT", [128, 128], bf16, kind="ExternalInput")
    WqPT = nc.dram_tensor("WqPT", [128, 128], bf16, kind="ExternalInput")
    WkT = nc.dram_tensor("WkT", [128, 128], bf16, kind="ExternalInput")
    WvT = nc.dram_tensor("WvT", [128, 128], bf16, kind="ExternalInput")
    W1T = nc.dram_tensor("W1T", [128, 128], bf16, kind="ExternalInput")
    bq_c = nc.dram_tensor("bq_c", [128, 1], dt, kind="ExternalInput")
    bk_c = nc.dram_tensor("bk_c", [128, 1], dt, kind="ExternalInput")
    bv_rep = nc.dram_tensor("bv_rep", [128, 128], dt, kind="ExternalInput")
    b1_r4 = nc.dram_tensor("b1_r4", [1, 4 * 128], bf16, kind="ExternalInput")
    W2_rep = nc.dram_tensor("W2_rep", [128, 128], bf16, kind="ExternalInput")
    b2_t = nc.dram_tensor("b2_t", [1, 1], dt, kind="ExternalInput")
    if ln_general:
        g_rep_t = nc.dram_tensor("g_rep", [128, 128], dt, kind="ExternalInput")
        lb_rep_t = nc.dram_tensor("lb_rep", [128, 128], dt, kind="ExternalInput")
    Y = nc.dram_tensor("Y", [BPC * H * TC, 128], dt, kind="ExternalOutput")

    with tile.TileContext(nc) as tc:
        _emit(nc, tc, locals(), ln_general)
    return nc


def _emit(nc, tc, T, ln_general):
    dt = f32
    from contextlib import ExitStack

    ctxmgr = ExitStack()
    with ctxmgr:
        singles = ctxmgr.enter_context(tc.tile_pool(name="singles", bufs=1))
        sb_hc = ctxmgr.enter_context(tc.tile_pool(name="hc", bufs=1))
        sb_ht = ctxmgr.enter_context(tc.tile_pool(name="ht", bufs=2))
        sb_kt = ctxmgr.enter_context(tc.tile_pool(name="kt", bufs=2))
        sb_v1a = ctxmgr.enter_context(tc.tile_pool(name="v1a", bufs=3))
        sb_w = ctxmgr.enter_context(tc.tile_pool(name="wp", bufs=3))
        sb_vp = ctxmgr.enter_context(tc.tile_pool(name="vp", bufs=5))
        sb_ut = ctxmgr.enter_context(tc.tile_pool(name="ut", bufs=5))
        sb_x1 = ctxmgr.enter_context(tc.tile_pool(name="x1", bufs=3))
        sb_x1tg = ctxmgr.enter_context(tc.tile_pool(name="x1tg", bufs=4))
        sb_x3g = ctxmgr.enter_context(tc.tile_pool(name="x3g", bufs=4))
        sb_scr = ctxmgr.enter_context(tc.tile_pool(name="scr", bufs=3))
        sb_stats = ctxmgr.enter_context(tc.tile_pool(name="stats", bufs=3))
        # PSUM: 3 (U halves) + 3 (x2 groups) + 2 (shared scratch) = 8 banks
        ps_u = ctxmgr.enter_context(tc.tile_pool(name="psu", bufs=3, space="PSUM"))
        ps_x2 = ctxmgr.enter_context(tc.tile_pool(name="psx", bufs=3, space="PSUM"))
        ps_b = ctxmgr.enter_context(tc.tile_pool(name="psb", bufs=2, space="PSUM"))

        # ---- load constants ----
        def load(name, shape, tdt=dt):
            t = singles.tile(shape, tdt, tag=name)
            nc.sync.dma_start(out=t[:], in_=T[name][:])
            return t

        at_s = load("AT", [128, BPC * H], bf16)
        phiT_s = load("phiT", [128, NTAR], bf16)
        wqat_s = load("WqAT", [128, 128], bf16)
        wqpt_s = load("WqPT", [128, 128], bf16)
        wkt_s = load("WkT", [128, 128], bf16)
        wvt_s = load("WvT", [128, 128], bf16)
        w1t_s = load("W1T", [128, 128], bf16)
        bq_s = load("bq_c", [128, 1])
        bk_s = load("bk_c", [128, 1])
        bv_s = load("bv_rep", [128, 128])
        b1r4_s = load("b1_r4", [1, 4 * 128], bf16)
        w2_s = load("W2_rep", [128, 128], bf16)
        if ln_general:
            g_s = load("g_rep", [128, 128])
            lb_s = load("lb_rep", [128, 128])
        b2_s = singles.tile([128, 1], dt, tag="b2")
        b2ap = T["b2_t"][:]
        nc.sync.dma_start(
            out=b2_s[:], in_=_bc(b2ap, [[0, 128], [1, 1]])
        )
        ident = singles.tile([128, 128], bf16, tag="ident")
        make_identity(nc, ident[:])
        identf = singles.tile([128, 128], dt, tag="identf")
        make_identity(nc, identf[:])
        eps_s = singles.tile([128, 1], dt, tag="eps")
        nc.vector.memset(eps_s[:], LN_EPS)
        ones1 = singles.tile([1, 128], bf16, tag="ones1")
        nc.vector.memset(ones1[:], 1.0)

        # front-load all H_emb chunk DMAs (overlap with Q setup matmuls)
        hchunks = {}
        for b in range(BPC):
            for oc in range(OC):
                hc = sb_hc.tile([128, 128], bf16, tag=f"hc{b}_{oc}")
                nc.sync.dma_start(
                    out=hc[:], in_=T["Hemb"][b, oc * 128 : (oc + 1) * 128, :]
                )
                hchunks[(b, oc)] = hc

        # ---- Qphi^T = WqP @ phi^T + bq : [j=128, t=256] ----
        qphi_ps = ps_b.tile([128, NTAR], dt, tag="ps")
        nc.tensor.matmul(qphi_ps[:], wqpt_s[:], phiT_s[:], start=True, stop=True)
        qphiT = singles.tile([128, NTAR], bf16, tag="qphiT")
        nc.scalar.add(out=qphiT[:], in_=qphi_ps[:], add=bq_s[:])

        # ---- QA^T : [j=128, (b,h)=24] ----
        qa_ps = ps_b.tile([128, BPC * H], dt, tag="ps")
        nc.tensor.matmul(qa_ps[:], wqat_s[:], at_s[:], start=True, stop=True)
        qaT = singles.tile([128, BPC * H], bf16, tag="qaT")
        nc.scalar.copy(out=qaT[:], in_=qa_ps[:])

        # ---- block-diagonal (head-masked) Qphi / QA so the per-head K=32
        # score contractions become full-K=128 matmuls (avoids row-packed
        # tile_position matmuls, which fault on concurrent same-bank psum
        # writes here) ----
        qblk = singles.tile([128, NH, NTAR], bf16, tag="qblk")
        nc.vector.memset(qblk[:], 0.0)
        qablk = singles.tile([128, NH, BPC * H], bf16, tag="qablk")
        nc.vector.memset(qablk[:], 0.0)
        for n in range(NH):
            nc.vector.tensor_copy(
                out=qblk[32 * n : 32 * (n + 1), n, :],
                in_=qphiT[32 * n : 32 * (n + 1), :],
            )
            nc.vector.tensor_copy(
                out=qablk[32 * n : 32 * (n + 1), n, :],
                in_=qaT[32 * n : 32 * (n + 1), :],
            )

        y_all = singles.tile([128, BPC, H, TC], dt, tag="y_all")

        for b in range(BPC):
            # ---- H^T via PE transpose: [c=128, o=512], one wide evac ----
            htb = sb_ht.tile([128, NOBS], bf16, tag="htb")
            ht_ps = ps_b.tile([128, OC, 128], bf16, tag="ps")
            for oc in range(OC):
                nc.tensor.transpose(
                    ht_ps[:, oc, :], hchunks[(b, oc)][:], ident[:]
                )
            nc.scalar.copy(
                out=htb[:].rearrange("p (g c) -> p g c", g=OC), in_=ht_ps[:]
            )

            # ---- K^T = Wk @ H^T + bk : [j=128, o=512] ----
            kt_ps = ps_b.tile([128, NOBS], dt, tag="ps")
            nc.tensor.matmul(kt_ps[:], wkt_s[:], htb[:], start=True, stop=True)
            ktb = sb_kt.tile([128, NOBS], bf16, tag="ktb")
            nc.scalar.add(out=ktb[:], in_=kt_ps[:], add=bk_s[:])

            # ---- per o-chunk: V, SA->w, Sphi->U, V' ----
            vp_tiles = []
            ut_tiles = []
            for oc in range(OC):
                # V chunk [o=128, j=128]
                v_ps = ps_b.tile([128, 128], dt, tag="ps")
                nc.tensor.matmul(
                    v_ps[:],
                    htb[:, oc * 128 : (oc + 1) * 128],
                    wvt_s[:],
                    start=True,
                    stop=True,
                )
                # V1a [o, n, d33]: (V + bv | 1)
                v1a = sb_v1a.tile([128, NH, DH + 1], bf16, tag="v1a")
                nc.vector.tensor_add(
                    out=v1a[:, :, 0:DH],
                    in0=v_ps[:].rearrange("p (n d) -> p n d", n=NH),
                    in1=bv_s[:].rearrange("p (n d) -> p n d", n=NH),
                )
                nc.vector.memset(v1a[:, :, DH : DH + 1], 1.0)

                # SA^T [o, (n,h)=48] : one K=128 matmul vs head-masked QA
                sa_ps = ps_b.tile([128, NH * H], dt, tag="ps")
                nc.tensor.matmul(
                    sa_ps[:],
                    ktb[:, oc * 128 : (oc + 1) * 128],
                    qablk[:, :, b * H : (b + 1) * H],
                    start=True,
                    stop=True,
                )
                w_oc = sb_w.tile([128, NH * H], bf16, tag="w_oc")
                nc.scalar.activation(
                    out=w_oc[:], in_=sa_ps[:], func=AF.Exp, scale=SCALE
                )

                # Sphi^T (-> U) [o, n, t=256] : two K=128 N=512 matmuls
                # into two 1-bank PSUM tiles; exp'd separately
                ut = sb_ut.tile([128, NH, NTAR], bf16, tag="ut")
                for i in range(2):
                    u_ps = ps_u.tile([128, 2, NTAR], dt, tag="psu")
                    nc.tensor.matmul(
                        u_ps[:],
                        ktb[:, oc * 128 : (oc + 1) * 128],
                        qblk[:, 2 * i : 2 * (i + 1), :],
                        start=True,
                        stop=True,
                    )
                    nc.scalar.activation(
                        out=ut[:, 2 * i : 2 * (i + 1), :],
                        in_=u_ps[:],
                        func=AF.Exp,
                        scale=SCALE,
                    )
                ut_tiles.append(ut)

                # V' [o, n, h, d33] = V1a * w (broadcast h / d33)
                vp = sb_vp.tile([128, NH, H, DH + 1], bf16, tag="vp")
                v1a_ap = v1a[:]
                in0 = _bc(
                    v1a_ap,
                    [v1a_ap.ap[0], [DH + 1, NH], [0, H], [1, DH + 1]],
                )
                w_ap = w_oc[:]
                in1 = _bc(
                    w_ap, [w_ap.ap[0], [H, NH], [1, H], [0, DH + 1]]
                )
                nc.gpsimd.tensor_mul(out=vp[:], in0=in0, in1=in1)
                vp_tiles.append(vp)

            # ---- ctx (numerator | denominator) + normalize -> x1 ----
            for tcc in range(TC):
                x1 = sb_x1.tile([128, H, NH, DH], bf16, tag="x1")
                for n in range(NH):
                    ctx_ps = ps_b.tile([128, H, DH + 1], dt, tag="ps")
                    for oc in range(OC):
                        nc.tensor.matmul(
                            ctx_ps[:],
                            ut_tiles[oc][:, n, tcc * 128 : (tcc + 1) * 128],
                            vp_tiles[oc][:, n, :, :],
                            start=(oc == 0),
                            stop=(oc == OC - 1),
                        )
                    rec = sb_stats.tile([128, H, 1], dt, tag="rec")
                    nc.vector.reciprocal(
                        out=rec[:], in_=ctx_ps[:, :, DH : DH + 1]
                    )
                    rec_ap = rec[:]
                    rec_bc = _bc(rec_ap, [rec_ap.ap[0], [1, H], [0, DH]])
                    nc.vector.tensor_mul(
                        out=x1[:, :, n, :], in0=ctx_ps[:, :, 0:DH], in1=rec_bc
                    )

                # ---- FFN in groups of 4 h: transpose -> wide evac ->
                # bias-preload MM + 4 W1 MMs -> grouped bn_stats ----
                NG = H // 4  # 3 groups
                stats_all = sb_stats.tile([128, H, 6], dt, tag="bnst")
                x2g_tiles = []
                for g in range(NG):
                    x1t_ps = ps_b.tile([128, 4, 128], bf16, tag="ps")
                    for j in range(4):
                        nc.tensor.transpose(
                            x1t_ps[:, j, :], x1[:, 4 * g + j, :, :], ident[:]
                        )
                    x1tg = sb_x1tg.tile([128, 4, 128], bf16, tag="x1tg")
                    nc.vector.tensor_copy(out=x1tg[:], in_=x1t_ps[:])

                    x2g = ps_x2.tile([128, 4, 128], dt, tag="psx")
                    for j in range(4):
                        nc.tensor.matmul(
                            x2g[:, j, :],
                            ones1[:],
                            b1r4_s[:, 128 * j : 128 * (j + 1)],
                            start=True,
                            stop=False,
                        )
                        nc.tensor.matmul(
                            x2g[:, j, :],
                            x1tg[:, j, :],
                            w1t_s[:],
                            start=False,
                            stop=True,
                        )
                    # per-h bn_stats (grouped form collapses APs and breaks)
                    for j in range(4):
                        nc.vector.bn_stats(
                            out=stats_all[:, 4 * g + j, :], in_=x2g[:, j, :]
                        )
                    x2g_tiles.append(x2g)

                # batched LN stats combine (even/odd halves -> mean/var)
                # var = (M2e + M2o)/128 + ((me - mo)/2)^2
                me = stats_all[:, :, 1]
                mo = stats_all[:, :, 4]
                ve = stats_all[:, :, 2]
                vo = stats_all[:, :, 5]
                msum = sb_stats.tile([128, H], dt, tag="msum")
                nc.vector.tensor_add(out=msum[:], in0=me, in1=mo)
                mdif = sb_stats.tile([128, H], dt, tag="mdif")
                nc.vector.tensor_sub(out=mdif[:], in0=me, in1=mo)
                m2s = sb_stats.tile([128, H], dt, tag="m2s")
                nc.vector.tensor_add(out=m2s[:], in0=ve, in1=vo)
                d2 = sb_stats.tile([128, H], dt, tag="d2")
                nc.vector.tensor_mul(out=d2[:], in0=mdif[:], in1=mdif[:])
                # var*128 = m2s + 32*d2 ; Rsqrt applies the 1/128 scale
                var = sb_stats.tile([128, H], dt, tag="var")
                nc.vector.scalar_tensor_tensor(
                    out=var[:], in0=d2[:], scalar=32.0,
                    in1=m2s[:], op0=ALU.mult, op1=ALU.add,
                )
                std = sb_stats.tile([128, H], dt, tag="std")
                nc.scalar.activation(
                    out=std[:], in_=var[:], func=AF.Sqrt,
                    scale=1.0 / 128.0, bias=eps_s[:],
                )
                rstd = sb_stats.tile([128, H], dt, tag="rstd")
                nc.vector.reciprocal(out=rstd[:], in_=std[:])
                nmr = sb_stats.tile([128, H], dt, tag="nmr")
                nc.vector.scalar_tensor_tensor(
                    out=nmr[:], in0=msum[:], scalar=-0.5, in1=rstd[:],
                    op0=ALU.mult, op1=ALU.mult,
                )

                for g in range(NG):
                    x2g = x2g_tiles[g]
                    x3g = sb_x3g.tile([128, 4, 128], bf16, tag="x3g")
                    for j in range(4):
                        h = 4 * g + j
                        if not ln_general:
                            nc.scalar.activation(
                                out=x3g[:, j, :],
                                in_=x2g[:, j, :],
                                func=AF.Relu,
                                scale=rstd[:, h : h + 1],
                                bias=nmr[:, h : h + 1],
                            )
                        else:
                            xn = sb_x3g.tile([128, 128], dt, tag="xn")
                            nc.scalar.activation(
                                out=xn[:],
                                in_=x2g[:, j, :],
                                func=AF.Identity,
                                scale=rstd[:, h : h + 1],
                                bias=nmr[:, h : h + 1],
                            )
                            nc.vector.tensor_mul(
                                out=xn[:], in0=xn[:], in1=g_s[:]
                            )
                            nc.vector.tensor_add(
                                out=xn[:], in0=xn[:], in1=lb_s[:]
                            )
                            nc.scalar.activation(
                                out=x3g[:, j, :], in_=xn[:], func=AF.Relu
                            )
                    # grouped W2 dot: scr = x3g * w2 ; reduce over o
                    scr = sb_scr.tile([128, 4, 128], bf16, tag="scr")
                    w2_ap = w2_s[:]
                    nc.gpsimd.tensor_mul(
                        out=scr[:],
                        in0=x3g[:],
                        in1=_bc(w2_ap, [w2_ap.ap[0], [0, 4], [1, 128]]),
                    )
                    nc.vector.tensor_reduce(
                        out=y_all[:, b, 4 * g : 4 * (g + 1), tcc : tcc + 1],
                        in_=scr[:],
                        axis=mybir.AxisListType.X,
                        op=ALU.add,
                    )

        # ---- finalize y: +b2, transpose, DMA out ----
        nc.vector.tensor_scalar_add(out=y_all[:], in0=y_all[:], scalar1=b2_s[:])
        y_ps = ps_b.tile([BPC * H * TC, 128], dt, tag="ps")
        nc.tensor.transpose(y_ps[:], y_all[:], identf[:])
        yT = singles.tile([BPC * H * TC, 128], dt, tag="yT")
        nc.scalar.copy(out=yT[:], in_=y_ps[:])
        nc.sync.dma_start(out=T["Y"][:], in_=yT[:])


# ------------------------------------------------------------------
def prepare_in_maps(inputs):
    import ml_dtypes

    bf = ml_dtypes.bfloat16
    A = np.ascontiguousarray(inputs["A"], np.float32)
    phi = np.ascontiguousarray(inputs["phi_tar"], np.float32)
    Hm = np.ascontiguousarray(inputs["H_emb_obs"], np.float32)
    Wq = np.asarray(inputs["Wq"], np.float32)
    Wk = np.asarray(inputs["Wk"], np.float32)
    Wv = np.asarray(inputs["Wv"], np.float32)
    W1 = np.asarray(inputs["W1"], np.float32)
    W2 = np.asarray(inputs["W2"], np.float32)

    shared = {
        "phiT": np.ascontiguousarray(phi.T.astype(bf)),
        "WqAT": np.ascontiguousarray(Wq[:, :DTOT].T.astype(bf)),
        "WqPT": np.ascontiguousarray(Wq[:, DTOT:].T.astype(bf)),
        "WkT": np.ascontiguousarray(Wk.T.astype(bf)),
        "WvT": np.ascontiguousarray(Wv.T.astype(bf)),
        "W1T": np.ascontiguousarray(W1.T.astype(bf)),
        "bq_c": np.ascontiguousarray(
            np.asarray(inputs["bq"], np.float32).reshape(128, 1)
        ),
        "bk_c": np.ascontiguousarray(
            np.asarray(inputs["bk"], np.float32).reshape(128, 1)
        ),
        "bv_rep": np.ascontiguousarray(
            np.broadcast_to(np.asarray(inputs["bv"], np.float32), (128, 128))
        ),
        "b1_r4": np.ascontiguousarray(
            np.tile(np.asarray(inputs["b1"], np.float32), 4)[None, :].astype(bf)
        ),
        "W2_rep": np.ascontiguousarray(
            np.broadcast_to(W2[0], (128, 128)).astype(bf)
        ),
        "b2_t": np.ascontiguousarray(
            np.asarray(inputs["b2"], np.float32).reshape(1, 1)
        ),
    }
    ln_g = np.asarray(inputs["ln_g"], np.float32)
    ln_b = np.asarray(inputs["ln_b"], np.float32)
    ln_general = not (
        np.allclose(ln_g, 1.0, atol=1e-7) and np.allclose(ln_b, 0.0, atol=1e-7)
    )
    if ln_general:
        shared["g_rep"] = np.ascontiguousarray(np.broadcast_to(ln_g, (128, 128)))
        shared["lb_rep"] = np.ascontiguousarray(np.broadcast_to(ln_b, (128, 128)))

    in_maps = []
    for k in range(NCORES):
        sl = slice(k * BPC, (k + 1) * BPC)
        m = dict(shared)
        m["AT"] = np.ascontiguousarray(
            A[sl].transpose(2, 0, 1).reshape(128, BPC * H).astype(bf)
        )
        m["Hemb"] = np.ascontiguousarray(Hm[sl].astype(bf))
        in_maps.append(m)
    return in_maps, ln_general


_nc_cache = {}
last_exec_time_ns = None
last_trace_path = None


def get_nc(ln_general: bool):
    if ln_general not in _nc_cache:
        _nc_cache[ln_general] = build_nc(ln_general)
    return _nc_cache[ln_general]


def kernel(**inputs) -> np.ndarray:
    global last_exec_time_ns, last_trace_path
    in_maps, ln_general = prepare_in_maps(inputs)
    nc = get_nc(ln_general)
    trace = bool(int(os.environ.get("KERNEL_TRACE", "0")))
    res = run_bass_kernel_spmd(
        nc, in_maps, core_ids=list(range(NCORES)), trace=trace
    )
    last_exec_time_ns = res.exec_time_ns
    if res.instructions_and_trace is not None:
        last_trace_path = res.instructions_and_trace[1]
    out = np.empty((B, H, NTAR, 1), np.float32)
    for k in range(NCORES):
        yk = res.results[k]["Y"].reshape(BPC, H, TC * 128)
        out[k * BPC : (k + 1) * BPC, :, :, 0] = yk
    return out

